# revision 17
# baseline (speedup 1.0000x reference)
"""Trainium2 kernel for residual-bit-quantized batch-ensemble Conv2d.

Problem: x (160,128,32,32) f32; U (5,147456) f32.
  w = 2-step residual quantization of U  -> (640,128,3,3) conv weight
  out[b] = conv2d(x[b], w[b % 5], stride 1, pad 1)   (grouped batch-ensemble)

Key facts exploited:
  * The quantized weight is exactly w = s2 * m with m integer in [-8, 7]
    (m = 5*floor(U/s1) + floor((U-v1)/s2)).  m is exactly representable in
    fp16, so the only precision loss is rounding x to fp16 (~2e-4 rel).
  * fp16 matmuls run at full PE rate (1 cycle/row), unlike fp32 (4x).
  * IN_CH = OUT_CH = 128 = partition count: conv becomes 9 accumulating
    (128x128) @ (128x512) matmuls per half-image into one PSUM bank.

Sharding: data-parallel over the batch: core c gets images [20c, 20c+20).
Weight quantization is done on host (numpy fp32, bit-identical to the
reference computation) - it is 0.01% of the FLOPs.

Schedule (per core): images processed one at a time (member-major), so the
first matmuls only wait for one 296KB image DMA. Three DMA streams: x on
the Sync HWDGE queue, w on the GpSimd SWDGE queue, y out on the Scalar
HWDGE queue. Junk warmup matmuls release the PE HAM clock-gate during the
DMA lead-in.
"""

import numpy as np

import concourse.bacc as bacc
import concourse.bass as bass
import concourse.mybir as mybir
import concourse.tile as tile
from concourse import bass_utils

# Problem constants (hardcoded per contract)
N_ENS = 5
C = 128          # input channels (= contraction dim = partitions)
O = 128          # output channels per ensemble member
H = W = 32
HP = WP = 34     # padded spatial
B = 160
N_CORES = 8
BPC = B // N_CORES          # 20 images per core
JPM = BPC // N_ENS          # 4 images per member per core
NHALF = 2                   # each image split into 2 row-halves of 16 rows
NPIX = 512                  # 16*32 output pixels per half = 1 PSUM bank of f32

F16 = mybir.dt.float16
F32 = mybir.dt.float32

N_WARM = 62


def _quantize_host(U: np.ndarray):
    """Residual-bit quantization, numerically identical to the jax reference
    (verified bitwise).  Returns integer weight m (as f32) and scale s2."""
    U = np.ascontiguousarray(U, dtype=np.float32)
    beta = np.max(U)
    alpha = np.min(U)
    s1 = np.float32((beta - alpha) / np.float32(3.0))
    f1 = np.floor(U / s1).astype(np.float32)
    v1 = (s1 * f1).astype(np.float32)
    s2 = np.float32(s1 / np.float32(5.0))
    f2 = np.floor(((U - v1) / s2).astype(np.float32)).astype(np.float32)
    m = np.float32(5.0) * f1 + f2          # exact small integers
    return m, s2


def _build_nc(s2: float):
    """Build the per-core Bass program (SPMD: same program on all 8 cores)."""
    nc = bacc.Bacc("TRN2", target_bir_lowering=False, debug=False)

    # Per-core inputs
    xs = nc.dram_tensor("xs", (N_ENS, JPM, C, HP, WP), F16, kind="ExternalInput")
    wt = nc.dram_tensor("wt", (N_ENS, C, 9 * O), F16, kind="ExternalInput")
    y = nc.dram_tensor("y", (N_ENS, JPM, O, NHALF, NPIX), F32, kind="ExternalOutput")

    with tile.TileContext(nc) as tc:
        with (
            tc.tile_pool(name="wpool", bufs=N_ENS) as wpool,
            tc.tile_pool(name="xpool", bufs=BPC) as xpool,
            tc.tile_pool(name="opool", bufs=4) as opool,
            tc.tile_pool(name="zpool", bufs=1) as zpool,
            tc.tile_pool(name="psum", bufs=4, space=bass.MemorySpace.PSUM) as pp,
        ):
            # x DMAs on the Sync HWDGE queue, in processing order so the
            # first image lands first.
            # Critical path: the FIRST matmul needs only w0[k=0] (33KB) and
            # x00 (296KB).  Member 0's weights are 9 separate per-k tiles so
            # Tile's (per-tile) dependency tracking lets matmul k start as
            # soon as its own 33KB slice lands.  Queue plan: sync = w0k0,
            # x00a, w0k1..8, x01..; scalar = x00b; gpsimd = w1..w4 whole.
            w0tiles = [
                wpool.tile([C, O], F16, tag="w0k", name=f"w0k_{k}", bufs=9)
                for k in range(9)
            ]
            wtiles = [None] + [
                wpool.tile([C, 9 * O], F16, tag="wt", name=f"w_{n}", bufs=4)
                for n in range(1, N_ENS)
            ]
            w0d = wt[0].rearrange("c (k o) -> c k o", o=O)
            nc.sync.dma_start(w0tiles[0][:], w0d[:, 0, :])
            xtiles = {}
            x00 = xpool.tile([C, HP, WP], F16, tag="xt", name="x_0_0")
            nc.sync.dma_start(x00[:, :HP // 2, :], xs[0, 0, :, :HP // 2, :])
            nc.scalar.dma_start(x00[:, HP // 2:, :], xs[0, 0, :, HP // 2:, :])
            xtiles[(0, 0)] = x00
            for k in range(1, 9):
                nc.sync.dma_start(w0tiles[k][:], w0d[:, k, :])
            for n in range(N_ENS):
                for j in range(JPM):
                    if (n, j) == (0, 0):
                        continue
                    xt = xpool.tile([C, HP, WP], F16, tag="xt", name=f"x_{n}_{j}")
                    nc.sync.dma_start(xt[:], xs[n, j, :, :, :])
                    xtiles[(n, j)] = xt
            for n in range(1, N_ENS):
                nc.gpsimd.dma_start(wtiles[n][:], wt[n, :, :])

            # PE warmup: junk matmuls while the first DMAs land, so the HAM
            # clock-gate releases (1.2 -> 2.4 GHz) before the real stream.
            # memset on the (idle) vector engine - gpsimd is busy issuing
            # the w DMA descriptors and would delay the warmup by ~4us.
            wz = zpool.tile([C, 64], F16, tag="wz", name="wz")
            nc.vector.memset(wz[:], 0.0)
            warm_ps = pp.tile([O, NHALF, NPIX], F32, tag="ps", name="warm_ps")
            for i in range(N_WARM):
                nc.tensor.matmul(
                    warm_ps[:32, 0, :64], wz[:, :32], wz[:, :],
                    start=True, stop=True,
                )

            img = 0
            for n in range(N_ENS):
                for j in range(JPM):
                    xt = xtiles[(n, j)]
                    ps = pp.tile([O, NHALF, NPIX], F32, tag="ps", name=f"ps_{n}_{j}")
                    for k in range(9):
                        kh, kw = divmod(k, 3)
                        if n == 0:
                            wk = w0tiles[k][:, :]              # (128c, 128o)
                        else:
                            wk = wtiles[n][:, k * O:(k + 1) * O]
                        for h in range(NHALF):
                            rhs = xt[:, 16 * h + kh:16 * h + kh + 16, kw:kw + W]
                            nc.tensor.matmul(
                                ps[:, h, :],
                                wk,
                                rhs,
                                start=(k == 0),
                                stop=(k == 8),
                            )
                    ot = opool.tile([O, NHALF, NPIX], F32, tag="ot")
                    last = (img == BPC - 1)
                    if last:
                        # last image is pure tail: split the copy across both
                        # engines and the store across both HWDGE queues
                        nc.scalar.mul(ot[:, 0, :], ps[:, 0, :], float(s2))
                        nc.vector.tensor_scalar_mul(ot[:, 1, :], ps[:, 1, :], float(s2))
                        nc.sync.dma_start(y[n, j, :, 0, :], ot[:, 0, :])
                        nc.scalar.dma_start(y[n, j, :, 1, :], ot[:, 1, :])
                    else:
                        # alternate engines so copies overlap
                        if img % 2 == 0:
                            nc.scalar.mul(ot[:], ps[:], float(s2))
                        else:
                            nc.vector.tensor_scalar_mul(ot[:], ps[:], float(s2))
                        # alternate output queues to halve store backlog
                        eng = nc.scalar if img % 2 == 0 else nc.sync
                        eng.dma_start(y[n, j, :, :, :], ot[:])
                    img += 1

    nc.compile()
    return nc


def run(x: np.ndarray, U: np.ndarray, **spmd_kwargs):
    m, s2 = _quantize_host(U)

    # Weight layout: U[n] -> (oc, ic, kh, kw); device wants [n][ic][k*O+oc]
    m5 = m.reshape(N_ENS, O, C, 3, 3)
    wt_host = np.ascontiguousarray(
        m5.transpose(0, 2, 3, 4, 1).reshape(N_ENS, C, 9 * O).astype(np.float16)
    )

    # x: pad to 34x34, cast fp16, regroup [core][member n][j][ch][hp][wp]
    xp = np.zeros((B, C, HP, WP), np.float16)
    xp[:, :, 1:1 + H, 1:1 + W] = x.astype(np.float16)
    # image index within a core: i = 5*j + n
    xg = xp.reshape(N_CORES, JPM, N_ENS, C, HP, WP).transpose(0, 2, 1, 3, 4, 5)
    xg = np.ascontiguousarray(xg)

    nc = _build_nc(float(s2))
    in_maps = [{"xs": xg[c], "wt": wt_host} for c in range(N_CORES)]
    res = bass_utils.run_bass_kernel_spmd(
        nc, in_maps, core_ids=list(range(N_CORES)), **spmd_kwargs
    )

    out = np.empty((B, O, H, W), np.float32)
    for c in range(N_CORES):
        yc = np.asarray(res.results[c]["y"]).reshape(N_ENS, JPM, O, H, W)
        t = yc.transpose(1, 0, 2, 3, 4).reshape(BPC, O, H, W)
        out[BPC * c:BPC * (c + 1)] = t
    return out, res


def kernel(x: np.ndarray, U: np.ndarray) -> np.ndarray:
    out, _ = run(x, U)
    return out


if __name__ == "__main__":
    rng = np.random.default_rng(0)
    x = rng.standard_normal((B, C, H, W), dtype=np.float32)
    U = (rng.standard_normal((N_ENS, C * O * 9), dtype=np.float32)
         * np.sqrt(2.0 / (C * O * 9)).astype(np.float32))
    out = kernel(x, U)
    print("out", out.shape, out.dtype, float(np.abs(out).max()))
